# revision 13
# baseline (speedup 1.0000x reference)
"""Trainium2 Bass kernel for a CustomGRUCell.

reference:
    r = sigmoid(x @ W_ir.T + b_ir + h @ W_hr.T)
    z = sigmoid(x @ W_iz.T + b_iz + h @ W_hz.T)
    n = tanh(x @ W_in.T + b_in + (r * h) @ W_hn.T)
    h_t = (1 - z) * n + z * h
    returns (h_t, r, z, n)

Shapes: x,h [8192, 2048]; W_* [2048, 2048]; b_* [2048]. All float32.

Strategy: data-parallel over the batch dim (1024 rows per core, 8 cores),
weights replicated. All compute happens in the "transposed world":
the host packs x^T, h^T and W^T so the contraction dim (IN / H-col) lands
on SBUF partitions for both matmul operands; outputs come back as
gate^T [H, B_shard] and are untransposed on the host.

All matmul operands are float16 (same 1.0 cycles/row PE rate as fp32r on
TRN2, half the DMA bytes and SBUF footprint; quantization error ~3e-4 vs
the 2e-2 gate). PSUM accumulates in fp32. Gate outputs are written as
fp16 and upcast to fp32 on the host.

Everything is SBUF-resident: x^T/h^T (packed together as `xh`) stay
loaded for all three phases; r*h and n are produced into resident SBUF
buffers (no DRAM scratch round trips, no h reload for phase 3).

The schedule is tuned so the PE never waits after the startup region.
The startup bottleneck is DMA *descriptor dispatch* (~0.6us each,
serialized on the sync queue; engines that can also trigger DMAs don't
wake any earlier and lengthen the boot preamble). So the host packs the
phase-1 head-block weights (`hw`) and the activations (`xh`) in exactly
the order the PE consumes them, letting a handful of descriptors deliver
whole ko-blocks:
- phase-1 head block covers BLK1=4 m-tiles (all 8 PSUM banks) with the
  ko-loop OUTER, consuming xh/hw chunks in arrival order; a big head
  block maximizes matmuls per streamed xh byte while DMA is still cold.
- each head unit's PSUM bank is consumed inline right after its final
  accumulation so the tail's first matmul finds a free bank.
- remaining m-tiles stream full-width weight tiles two per m-tile.
"""

import numpy as np

import concourse.bass as bass
import concourse.bacc as bacc
import concourse.mybir as mybir
import concourse.tile as tile
from concourse.bass_utils import run_bass_kernel_spmd

F16 = mybir.dt.float16
F32 = mybir.dt.float32
AFT = mybir.ActivationFunctionType

# Problem constants (hardcoded per contract).
B_FULL = 8192
IN = 2048
H = 2048
N_CORES = 8
BS = B_FULL // N_CORES  # 1024 batch rows per core
P = 128
KO_IN = IN // P  # 16 contraction subtiles for x-gemms
KO_H = H // P    # 16 contraction subtiles for h/rh-gemms
MT = H // P      # 16 output row tiles
NFREE = 512      # moving free dim per matmul (1 PSUM bank of fp32)
NB = BS // NFREE  # 2 batch chunks per core
BLK1 = 4         # phase-1 head-block m-tiles (uses all 8 PSUM banks)
BLK = 3          # phase-2/3 head-block m-tiles

# Set by the test harness to capture an NTFF profile.
TRACE = False
LAST_RESULTS = None


def _build_nc():
    nc = bacc.Bacc("TRN2", target_bir_lowering=False, debug=False)

    # xh: x^T and h^T interleaved per ko in PE consumption order.
    xh = nc.dram_tensor(
        "xh", [P, KO_IN, 2, BS], F16, kind="ExternalInput").ap()
    # hw: phase-1 head-block weights, chunk-major by ko, (mt, a|b) minor.
    hw = nc.dram_tensor(
        "hw", [P, KO_IN, 2 * BLK1, P], F16, kind="ExternalInput").ap()
    w = {
        name: nc.dram_tensor(name, [MT, P, KO_IN * P], F16, kind="ExternalInput").ap()
        for name in ("w_ir", "w_hr", "w_iz", "w_hz", "w_in", "w_hn")
    }
    b = {
        name: nc.dram_tensor(name, [P, MT], F32, kind="ExternalInput").ap()
        for name in ("b_ir", "b_iz", "b_in")
    }
    outs = {
        name: nc.dram_tensor(name, [MT, P, BS], F16, kind="ExternalOutput").ap()
        for name in ("rT", "zT", "nT", "htT")
    }

    with tile.TileContext(nc) as tc:
        with (
            tc.tile_pool(name="xhres", bufs=1) as xh_pool,
            tc.tile_pool(name="hwres", bufs=1) as hw_pool,
            tc.tile_pool(name="rhres", bufs=1) as rh_pool,
            tc.tile_pool(name="nres", bufs=1) as n_pool,
            tc.tile_pool(name="wstream", bufs=10) as w_pool,
            tc.tile_pool(name="gates", bufs=6) as g_pool,
            tc.tile_pool(name="bias", bufs=1) as b_pool,
            tc.tile_pool(name="psum", bufs=8, space="PSUM") as ps_pool,
        ):
            def w_tile(w_ap, mt, nm):
                t = w_pool.tile([P, KO_IN * P], F16, tag="w", name=nm)
                nc.sync.dma_start(t[:], w_ap[mt])
                return t

            # Residents.
            xh_sb = xh_pool.tile([P, KO_IN, 2, BS], F16, tag="xh")
            hw_sb = hw_pool.tile([P, KO_IN, 2 * BLK1, P], F16, tag="hw")
            n_sb = n_pool.tile([P, MT, BS], F16, tag="n")

            # Startup DMA, in first-use order; one descriptor delivers a
            # whole arrival-order block.
            nc.sync.dma_start(hw_sb[:, 0], hw[:, 0])
            nc.sync.dma_start(xh_sb[:, 0, 0, 0:NFREE], xh[:, 0, 0, 0:NFREE])
            nc.sync.dma_start(xh_sb[:, 0, 0, NFREE:], xh[:, 0, 0, NFREE:])
            nc.sync.dma_start(xh_sb[:, 0, 1, 0:NFREE], xh[:, 0, 1, 0:NFREE])
            nc.sync.dma_start(xh_sb[:, 0, 1, NFREE:], xh[:, 0, 1, NFREE:])
            nc.sync.dma_start(hw_sb[:, 1], hw[:, 1])
            nc.sync.dma_start(xh_sb[:, 1], xh[:, 1])
            nc.sync.dma_start(hw_sb[:, 2:4], hw[:, 2:4])
            nc.sync.dma_start(xh_sb[:, 2], xh[:, 2])
            nc.sync.dma_start(xh_sb[:, 3:6], xh[:, 3:6])
            nc.sync.dma_start(hw_sb[:, 4:8], hw[:, 4:8])
            nc.sync.dma_start(xh_sb[:, 6:11], xh[:, 6:11])
            nc.sync.dma_start(hw_sb[:, 8:], hw[:, 8:])
            nc.sync.dma_start(xh_sb[:, 11:], xh[:, 11:])

            bias_sb = {}
            for name in ("b_ir", "b_iz", "b_in"):
                t = b_pool.tile([P, MT], F32, tag=name)
                nc.sync.dma_start(t[:], b[name][:])
                bias_sb[name] = t

            def xa(ko, nbs):
                return xh_sb[:, ko, 0, nbs]

            def hb(ko, nbs):
                return xh_sb[:, ko, 1, nbs]

            def phase(blk, head_w_sel, wa_ap, wb_ap, rhs_a, rhs_b, consume,
                      interleave_ab):
                """Head block (first `blk` m-tiles): ko-loop OUTER so the PE
                consumes streaming operand chunks in DMA arrival order.
                interleave_ab pairs A/B at each ko (phase 1: x and h arrive
                interleaved); otherwise all A first. Each unit's PSUM bank
                is consumed inline right after its final accumulation.
                Remaining m-tiles: mt-wise, both batch chunks interleaved
                per weight chunk."""
                units = [(mt, nb) for mt in range(blk) for nb in range(NB)]
                ps = {
                    u: ps_pool.tile(
                        [P, NFREE], F32, tag="ps", name=f"ps_{u[0]}_{u[1]}")
                    for u in units
                }

                def mm(u, side, rhs, ko, start, stop):
                    mt, nb = u
                    nc.tensor.matmul(
                        ps[u][:],
                        head_w_sel(side, mt, ko),
                        rhs(ko, slice(nb * NFREE, (nb + 1) * NFREE)),
                        start=start,
                        stop=stop,
                    )

                if interleave_ab:
                    for ko in range(KO_IN):
                        last = ko == KO_H - 1
                        for mt in range(blk):
                            for nb in range(NB):
                                mm((mt, nb), 0, rhs_a, ko, ko == 0, False)
                            for nb in range(NB):
                                mm((mt, nb), 1, rhs_b, ko, False, last)
                                if last:
                                    consume(mt, nb, ps[(mt, nb)])
                else:
                    for ko in range(KO_IN):
                        for u in units:
                            mm(u, 0, rhs_a, ko, ko == 0, False)
                    for ko in range(KO_H):
                        last = ko == KO_H - 1
                        for u in units:
                            mm(u, 1, rhs_b, ko, False, last)
                            if last:
                                consume(*u, ps[u])

                # steady tail: mt-wise, nb pairs adjacent per weight chunk
                for mt in range(blk, MT):
                    wa_t = w_tile(wa_ap, mt, f"wa{mt}")
                    wb_t = w_tile(wb_ap, mt, f"wb{mt}")
                    ps_t = [
                        ps_pool.tile(
                            [P, NFREE], F32, tag="ps", name=f"ps_{mt}_{nb}")
                        for nb in range(NB)
                    ]
                    for ko in range(KO_IN):
                        for nb in range(NB):
                            nc.tensor.matmul(
                                ps_t[nb][:],
                                wa_t[:, ko * P:(ko + 1) * P],
                                rhs_a(ko, slice(nb * NFREE, (nb + 1) * NFREE)),
                                start=(ko == 0), stop=False,
                            )
                    for ko in range(KO_H):
                        for nb in range(NB):
                            nc.tensor.matmul(
                                ps_t[nb][:],
                                wb_t[:, ko * P:(ko + 1) * P],
                                rhs_b(ko, slice(nb * NFREE, (nb + 1) * NFREE)),
                                start=False, stop=(ko == KO_H - 1),
                            )
                    for nb in range(NB):
                        consume(mt, nb, ps_t[nb])

            # ---- phase 1: r = sigmoid(x@W_ir^T + b_ir + h@W_hr^T); rh = r*h
            rh_sb = rh_pool.tile([P, KO_H, BS], F16, tag="rh", name="rh")

            def consume_r(mt, nb, ps_t):
                nbs = slice(nb * NFREE, (nb + 1) * NFREE)
                r_t = g_pool.tile([P, NFREE], F16, tag="g", name="r_t")
                nc.scalar.activation(
                    r_t[:], ps_t[:], AFT.Sigmoid,
                    bias=bias_sb["b_ir"][:, mt:mt + 1],
                )
                nc.sync.dma_start(outs["rT"][mt][:, nbs], r_t[:])
                nc.vector.tensor_mul(
                    rh_sb[:, mt, nbs], r_t[:], xh_sb[:, mt, 1, nbs])

            phase(
                BLK1,
                lambda s, mt, ko: hw_sb[:, ko, 2 * mt + s, :],
                w["w_ir"], w["w_hr"], xa, hb, consume_r, True)

            # ---- phase 2: n = tanh(x@W_in^T + b_in + rh@W_hn^T)
            pre2a = {mt: w_tile(w["w_in"], mt, f"wa{mt}") for mt in range(BLK)}
            pre2b = {mt: w_tile(w["w_hn"], mt, f"wb{mt}") for mt in range(BLK)}

            def consume_n(mt, nb, ps_t):
                nbs = slice(nb * NFREE, (nb + 1) * NFREE)
                nc.scalar.activation(
                    n_sb[:, mt, nbs], ps_t[:], AFT.Tanh,
                    bias=bias_sb["b_in"][:, mt:mt + 1],
                )
                nc.sync.dma_start(outs["nT"][mt][:, nbs], n_sb[:, mt, nbs])

            def pre_sel(pa, pb):
                return lambda s, mt, ko: (pa if s == 0 else pb)[mt][
                    :, ko * P:(ko + 1) * P]

            phase(
                BLK, pre_sel(pre2a, pre2b), w["w_in"], w["w_hn"],
                xa, lambda ko, nbs: rh_sb[:, ko, nbs], consume_n, False)

            # ---- phase 3: z = sigmoid(x@W_iz^T + b_iz + h@W_hz^T)
            #      h_t = n + z*(h - n)
            pre3a = {mt: w_tile(w["w_iz"], mt, f"wa{mt}") for mt in range(BLK)}
            pre3b = {mt: w_tile(w["w_hz"], mt, f"wb{mt}") for mt in range(BLK)}

            # d = h - n precomputed on the (otherwise idle) DVE while the
            # phase-3 matmuls run; shortens the critical consume chain to
            # mul+add. Reuses the rh buffer (rh's last readers are phase-2
            # matmuls; the WAR hazard is tracked by the tile framework).
            d_sb = rh_pool.tile([P, MT, BS], F16, tag="rh", name="d")
            for mt in range(MT):
                for nb in range(NB):
                    nbs = slice(nb * NFREE, (nb + 1) * NFREE)
                    nc.vector.tensor_sub(
                        d_sb[:, mt, nbs], xh_sb[:, mt, 1, nbs],
                        n_sb[:, mt, nbs])

            def consume_z(mt, nb, ps_t):
                nbs = slice(nb * NFREE, (nb + 1) * NFREE)
                z_t = g_pool.tile([P, NFREE], F16, tag="g", name="z_t")
                nc.scalar.activation(
                    z_t[:], ps_t[:], AFT.Sigmoid,
                    bias=bias_sb["b_iz"][:, mt:mt + 1],
                )
                nc.sync.dma_start(outs["zT"][mt][:, nbs], z_t[:])
                zd_t = g_pool.tile([P, NFREE], F16, tag="g", name="zd_t")
                nc.vector.tensor_mul(zd_t[:], z_t[:], d_sb[:, mt, nbs])
                ht_t = g_pool.tile([P, NFREE], F16, tag="g", name="ht_t")
                nc.vector.tensor_add(ht_t[:], n_sb[:, mt, nbs], zd_t[:])
                nc.sync.dma_start(outs["htT"][mt][:, nbs], ht_t[:])

            phase(
                BLK, pre_sel(pre3a, pre3b), w["w_iz"], w["w_hz"],
                xa, hb, consume_z, False)

    nc.finalize()
    return nc


_NC = None


def _get_nc():
    global _NC
    if _NC is None:
        _NC = _build_nc()
    return _NC


def _pack_w(W):
    # W [H, IN] -> [MT, P, KO*P] with W_host[mt, p, ko, m] = W[mt*P+m, ko*P+p]
    W = np.ascontiguousarray(np.asarray(W, dtype=np.float32))
    return np.ascontiguousarray(
        W.reshape(MT, P, KO_IN, P).transpose(0, 3, 2, 1).astype(np.float16)
    ).reshape(MT, P, KO_IN * P)


def _pack_hw(pwa, pwb):
    # packed weights [MT, P, KO*P] x2 -> [P, KO, 2*BLK1, P] in arrival order
    st = np.stack(
        [pwa[:BLK1].reshape(BLK1, P, KO_IN, P),
         pwb[:BLK1].reshape(BLK1, P, KO_IN, P)], axis=1
    )  # [BLK1, 2, P, KO, P]
    return np.ascontiguousarray(st.transpose(2, 3, 0, 1, 4)).reshape(
        P, KO_IN, 2 * BLK1, P)


def _pack_xh(x, h):
    # x,h [BS, D] -> [P, KO, 2, BS] with xh[p, ko, i, b] = (x,h)[b, ko*P+p]
    def t(a):
        return (np.asarray(a, dtype=np.float32).reshape(BS, -1, P)
                .transpose(2, 1, 0).astype(np.float16))
    return np.ascontiguousarray(np.stack([t(x), t(h)], axis=2))


def _pack_b(bvec):
    # b [H] -> [P, MT] with b_host[p, mt] = b[mt*P+p]
    return np.ascontiguousarray(
        np.asarray(bvec, dtype=np.float32).reshape(MT, P).T
    )


def _unpack(arr):
    # [MT, P, BS] fp16 -> [BS, H] fp32
    return np.ascontiguousarray(
        arr.astype(np.float32).transpose(2, 0, 1)
    ).reshape(BS, H)


def kernel(x, h, W_ir, b_ir, W_hr, W_iz, b_iz, W_hz, W_in, b_in, W_hn):
    global LAST_RESULTS
    nc = _get_nc()

    x = np.ascontiguousarray(np.asarray(x, dtype=np.float32))
    h = np.ascontiguousarray(np.asarray(h, dtype=np.float32))

    pw = {
        "w_ir": _pack_w(W_ir), "w_hr": _pack_w(W_hr),
        "w_iz": _pack_w(W_iz), "w_hz": _pack_w(W_hz),
        "w_in": _pack_w(W_in), "w_hn": _pack_w(W_hn),
    }
    shared = {
        **pw,
        "hw": _pack_hw(pw["w_ir"], pw["w_hr"]),
        "b_ir": _pack_b(b_ir), "b_iz": _pack_b(b_iz), "b_in": _pack_b(b_in),
    }
    in_maps = []
    for c in range(N_CORES):
        sl = slice(c * BS, (c + 1) * BS)
        in_maps.append({
            "xh": _pack_xh(x[sl], h[sl]),
            **shared,
        })

    res = run_bass_kernel_spmd(
        nc, in_maps, core_ids=list(range(N_CORES)), trace=TRACE
    )
    LAST_RESULTS = res

    def full(name):
        return np.concatenate(
            [_unpack(res.results[c][name]) for c in range(N_CORES)], axis=0
        )

    return full("htT"), full("rT"), full("zT"), full("nT")


# revision 17
# speedup vs baseline: 1.0028x; 1.0028x over previous
"""Trainium2 Bass kernel for a CustomGRUCell.

reference:
    r = sigmoid(x @ W_ir.T + b_ir + h @ W_hr.T)
    z = sigmoid(x @ W_iz.T + b_iz + h @ W_hz.T)
    n = tanh(x @ W_in.T + b_in + (r * h) @ W_hn.T)
    h_t = (1 - z) * n + z * h
    returns (h_t, r, z, n)

Shapes: x,h [8192, 2048]; W_* [2048, 2048]; b_* [2048]. All float32.

Strategy: data-parallel over the batch dim (1024 rows per core, 8 cores),
weights replicated. All compute happens in the "transposed world":
the host packs x^T, h^T and W^T so the contraction dim (IN / H-col) lands
on SBUF partitions for both matmul operands; outputs come back as
gate^T [H, B_shard] and are untransposed on the host.

All matmul operands are float16 (same 1.0 cycles/row PE rate as fp32r on
TRN2, half the DMA bytes and SBUF footprint; quantization error ~3e-4 vs
the 2e-2 gate). PSUM accumulates in fp32. Gate outputs are written as
fp16 and upcast to fp32 on the host.

Everything is SBUF-resident: x^T/h^T (packed together as `xh`) stay
loaded for all three phases; r*h and n are produced into resident SBUF
buffers (no DRAM scratch round trips, no h reload for phase 3).

The schedule is tuned so the PE never waits after the startup region.
The startup bottleneck is DMA *descriptor dispatch* (~0.6us each,
serialized on the sync queue; engines that can also trigger DMAs don't
wake any earlier and lengthen the boot preamble). So the host packs the
phase-1 head-block weights (`hw`) and the activations (`xh`) in exactly
the order the PE consumes them, letting a handful of descriptors deliver
whole ko-blocks:
- phase-1 head block covers BLK1=4 m-tiles (all 8 PSUM banks) with the
  ko-loop OUTER, consuming xh/hw chunks in arrival order; a big head
  block maximizes matmuls per streamed xh byte while DMA is still cold.
- each head unit's PSUM bank is consumed inline right after its final
  accumulation so the tail's first matmul finds a free bank.
- remaining m-tiles stream full-width weight tiles two per m-tile.
"""

import numpy as np

import concourse.bass as bass
import concourse.bacc as bacc
import concourse.mybir as mybir
import concourse.tile as tile
from concourse.bass_utils import run_bass_kernel_spmd

F16 = mybir.dt.float16
F32 = mybir.dt.float32
AFT = mybir.ActivationFunctionType

# Problem constants (hardcoded per contract).
B_FULL = 8192
IN = 2048
H = 2048
N_CORES = 8
BS = B_FULL // N_CORES  # 1024 batch rows per core
P = 128
KO_IN = IN // P  # 16 contraction subtiles for x-gemms
KO_H = H // P    # 16 contraction subtiles for h/rh-gemms
MT = H // P      # 16 output row tiles
NFREE = 512      # moving free dim per matmul (1 PSUM bank of fp32)
NB = BS // NFREE  # 2 batch chunks per core
BLK1 = 4         # phase-1 head-block m-tiles (uses all 8 PSUM banks)
BLK = 3          # phase-2/3 head-block m-tiles

# Set by the test harness to capture an NTFF profile.
TRACE = False
LAST_RESULTS = None


def _build_nc():
    nc = bacc.Bacc("TRN2", target_bir_lowering=False, debug=False)

    # xh: x^T and h^T interleaved per ko in PE consumption order.
    xh = nc.dram_tensor(
        "xh", [P, KO_IN, 2, BS], F16, kind="ExternalInput").ap()
    # hw: phase-1 head-block weights, chunk-major by ko, (mt, a|b) minor.
    hw = nc.dram_tensor(
        "hw", [P, KO_IN, 2 * BLK1, P], F16, kind="ExternalInput").ap()
    w = {
        name: nc.dram_tensor(name, [MT, P, KO_IN * P], F16, kind="ExternalInput").ap()
        for name in ("w_ir", "w_hr", "w_iz", "w_hz", "w_in", "w_hn")
    }
    b = {
        name: nc.dram_tensor(name, [P, MT], F32, kind="ExternalInput").ap()
        for name in ("b_ir", "b_iz", "b_in")
    }
    outs = {
        name: nc.dram_tensor(name, [MT, P, BS], F16, kind="ExternalOutput").ap()
        for name in ("rT", "zT", "nT", "htT")
    }

    with tile.TileContext(nc) as tc:
        with (
            tc.tile_pool(name="xhres", bufs=1) as xh_pool,
            tc.tile_pool(name="hwres", bufs=1) as hw_pool,
            tc.tile_pool(name="rhres", bufs=1) as rh_pool,
            tc.tile_pool(name="nres", bufs=1) as n_pool,
            tc.tile_pool(name="wstream", bufs=10) as w_pool,
            tc.tile_pool(name="gates", bufs=6) as g_pool,
            tc.tile_pool(name="bias", bufs=1) as b_pool,
            tc.tile_pool(name="psum", bufs=8, space="PSUM") as ps_pool,
        ):
            def w_tile(w_ap, mt, nm):
                t = w_pool.tile([P, KO_IN * P], F16, tag="w", name=nm)
                nc.sync.dma_start(t[:], w_ap[mt])
                return t

            # Residents.
            xh_sb = xh_pool.tile([P, KO_IN, 2, BS], F16, tag="xh")
            hw_sb = hw_pool.tile([P, KO_IN, 2 * BLK1, P], F16, tag="hw")
            n_sb = n_pool.tile([P, MT, BS], F16, tag="n")

            # PE warmup: the first real matmul can't start until its DMA
            # lands (~10.4us: 6.8us engine preamble + descriptor dispatch +
            # cold transfer), and the PE p-state ramp needs ~3us of
            # continuous busy to reach 2.4GHz — so without warmup the first
            # ~14 real matmuls run at 1.2GHz. Dummy matmuls on a memset
            # tile (no DMA dependency) burn that dead window and hand the
            # real stream a fully ramped PE.
            warm_t = g_pool.tile([P, NFREE], F16, tag="g", name="warm")
            nc.gpsimd.memset(warm_t[:], 0.0)
            ps_warm = ps_pool.tile([P, NFREE], F32, tag="ps", name="ps_warm")
            for _ in range(18):
                nc.tensor.matmul(
                    ps_warm[:, 0:256], warm_t[:, 0:P], warm_t[:, 0:256],
                    start=True, stop=True, skip_group_check=True,
                )

            # Startup DMA, in first-use order; one descriptor delivers a
            # whole arrival-order block.
            nc.sync.dma_start(hw_sb[:, 0], hw[:, 0])
            nc.sync.dma_start(xh_sb[:, 0, 0, 0:NFREE], xh[:, 0, 0, 0:NFREE])
            nc.sync.dma_start(xh_sb[:, 0, 0, NFREE:], xh[:, 0, 0, NFREE:])
            nc.sync.dma_start(xh_sb[:, 0, 1, 0:NFREE], xh[:, 0, 1, 0:NFREE])
            nc.sync.dma_start(xh_sb[:, 0, 1, NFREE:], xh[:, 0, 1, NFREE:])
            nc.sync.dma_start(hw_sb[:, 1], hw[:, 1])
            nc.sync.dma_start(xh_sb[:, 1], xh[:, 1])
            nc.sync.dma_start(hw_sb[:, 2:4], hw[:, 2:4])
            nc.sync.dma_start(xh_sb[:, 2], xh[:, 2])
            nc.sync.dma_start(xh_sb[:, 3:6], xh[:, 3:6])
            nc.sync.dma_start(hw_sb[:, 4:8], hw[:, 4:8])
            nc.sync.dma_start(xh_sb[:, 6:11], xh[:, 6:11])
            nc.sync.dma_start(hw_sb[:, 8:], hw[:, 8:])
            nc.sync.dma_start(xh_sb[:, 11:], xh[:, 11:])

            bias_sb = {}
            for name in ("b_ir", "b_iz", "b_in"):
                t = b_pool.tile([P, MT], F32, tag=name)
                nc.sync.dma_start(t[:], b[name][:])
                bias_sb[name] = t

            def xa(ko, nbs):
                return xh_sb[:, ko, 0, nbs]

            def hb(ko, nbs):
                return xh_sb[:, ko, 1, nbs]

            def phase(blk, head_w_sel, wa_ap, wb_ap, rhs_a, rhs_b, consume,
                      interleave_ab, seq_last=False):
                """Head block (first `blk` m-tiles): ko-loop OUTER so the PE
                consumes streaming operand chunks in DMA arrival order.
                interleave_ab pairs A/B at each ko (phase 1: x and h arrive
                interleaved); otherwise all A first. Each unit's PSUM bank
                is consumed inline right after its final accumulation.
                Remaining m-tiles: mt-wise, both batch chunks interleaved
                per weight chunk."""
                units = [(mt, nb) for mt in range(blk) for nb in range(NB)]
                ps = {
                    u: ps_pool.tile(
                        [P, NFREE], F32, tag="ps", name=f"ps_{u[0]}_{u[1]}")
                    for u in units
                }

                def mm(u, side, rhs, ko, start, stop):
                    mt, nb = u
                    nc.tensor.matmul(
                        ps[u][:],
                        head_w_sel(side, mt, ko),
                        rhs(ko, slice(nb * NFREE, (nb + 1) * NFREE)),
                        start=start,
                        stop=stop,
                    )

                if interleave_ab:
                    for ko in range(KO_IN):
                        last = ko == KO_H - 1
                        for mt in range(blk):
                            for nb in range(NB):
                                mm((mt, nb), 0, rhs_a, ko, ko == 0, False)
                            for nb in range(NB):
                                mm((mt, nb), 1, rhs_b, ko, False, last)
                                if last:
                                    consume(mt, nb, ps[(mt, nb)])
                else:
                    for ko in range(KO_IN):
                        for u in units:
                            mm(u, 0, rhs_a, ko, ko == 0, False)
                    for ko in range(KO_H):
                        last = ko == KO_H - 1
                        for u in units:
                            mm(u, 1, rhs_b, ko, False, last)
                            if last:
                                consume(*u, ps[u])

                # steady tail: mt-wise, nb pairs adjacent per weight chunk.
                # For the final m-tile of the final phase (seq_last), run the
                # batch chunks sequentially instead: chunk 0's consume chain
                # (ACT + DVE + output DMA) then hides under chunk 1's 32
                # matmuls, leaving only one consume chain after the last
                # matmul of the whole kernel.
                for mt in range(blk, MT):
                    wa_t = w_tile(wa_ap, mt, f"wa{mt}")
                    wb_t = w_tile(wb_ap, mt, f"wb{mt}")
                    nbs_groups = (
                        [[nb] for nb in range(NB)]
                        if (seq_last and mt == MT - 1) else [list(range(NB))]
                    )
                    for nbs_g in nbs_groups:
                        ps_t = {
                            nb: ps_pool.tile(
                                [P, NFREE], F32, tag="ps", name=f"ps_{mt}_{nb}")
                            for nb in nbs_g
                        }
                        for ko in range(KO_IN):
                            for nb in nbs_g:
                                nc.tensor.matmul(
                                    ps_t[nb][:],
                                    wa_t[:, ko * P:(ko + 1) * P],
                                    rhs_a(ko, slice(nb * NFREE,
                                                    (nb + 1) * NFREE)),
                                    start=(ko == 0), stop=False,
                                )
                        for ko in range(KO_H):
                            for nb in nbs_g:
                                nc.tensor.matmul(
                                    ps_t[nb][:],
                                    wb_t[:, ko * P:(ko + 1) * P],
                                    rhs_b(ko, slice(nb * NFREE,
                                                    (nb + 1) * NFREE)),
                                    start=False, stop=(ko == KO_H - 1),
                                )
                        for nb in nbs_g:
                            consume(mt, nb, ps_t[nb])

            # ---- phase 1: r = sigmoid(x@W_ir^T + b_ir + h@W_hr^T); rh = r*h
            rh_sb = rh_pool.tile([P, KO_H, BS], F16, tag="rh", name="rh")

            def consume_r(mt, nb, ps_t):
                nbs = slice(nb * NFREE, (nb + 1) * NFREE)
                r_t = g_pool.tile([P, NFREE], F16, tag="g", name="r_t")
                nc.scalar.activation(
                    r_t[:], ps_t[:], AFT.Sigmoid,
                    bias=bias_sb["b_ir"][:, mt:mt + 1],
                )
                nc.sync.dma_start(outs["rT"][mt][:, nbs], r_t[:])
                nc.vector.tensor_mul(
                    rh_sb[:, mt, nbs], r_t[:], xh_sb[:, mt, 1, nbs])

            phase(
                BLK1,
                lambda s, mt, ko: hw_sb[:, ko, 2 * mt + s, :],
                w["w_ir"], w["w_hr"], xa, hb, consume_r, True)

            # ---- phase 2: n = tanh(x@W_in^T + b_in + rh@W_hn^T)
            pre2a = {mt: w_tile(w["w_in"], mt, f"wa{mt}") for mt in range(BLK)}
            pre2b = {mt: w_tile(w["w_hn"], mt, f"wb{mt}") for mt in range(BLK)}

            def consume_n(mt, nb, ps_t):
                nbs = slice(nb * NFREE, (nb + 1) * NFREE)
                nc.scalar.activation(
                    n_sb[:, mt, nbs], ps_t[:], AFT.Tanh,
                    bias=bias_sb["b_in"][:, mt:mt + 1],
                )
                nc.sync.dma_start(outs["nT"][mt][:, nbs], n_sb[:, mt, nbs])

            def pre_sel(pa, pb):
                return lambda s, mt, ko: (pa if s == 0 else pb)[mt][
                    :, ko * P:(ko + 1) * P]

            phase(
                BLK, pre_sel(pre2a, pre2b), w["w_in"], w["w_hn"],
                xa, lambda ko, nbs: rh_sb[:, ko, nbs], consume_n, False)

            # ---- phase 3: z = sigmoid(x@W_iz^T + b_iz + h@W_hz^T)
            #      h_t = n + z*(h - n)
            pre3a = {mt: w_tile(w["w_iz"], mt, f"wa{mt}") for mt in range(BLK)}
            pre3b = {mt: w_tile(w["w_hz"], mt, f"wb{mt}") for mt in range(BLK)}

            # d = h - n precomputed on the (otherwise idle) DVE while the
            # phase-3 matmuls run; shortens the critical consume chain to
            # mul+add. Reuses the rh buffer (rh's last readers are phase-2
            # matmuls; the WAR hazard is tracked by the tile framework).
            d_sb = rh_pool.tile([P, MT, BS], F16, tag="rh", name="d")
            for mt in range(MT):
                for nb in range(NB):
                    nbs = slice(nb * NFREE, (nb + 1) * NFREE)
                    nc.vector.tensor_sub(
                        d_sb[:, mt, nbs], xh_sb[:, mt, 1, nbs],
                        n_sb[:, mt, nbs])

            def consume_z(mt, nb, ps_t):
                nbs = slice(nb * NFREE, (nb + 1) * NFREE)
                z_t = g_pool.tile([P, NFREE], F16, tag="g", name="z_t")
                nc.scalar.activation(
                    z_t[:], ps_t[:], AFT.Sigmoid,
                    bias=bias_sb["b_iz"][:, mt:mt + 1],
                )
                nc.sync.dma_start(outs["zT"][mt][:, nbs], z_t[:])
                zd_t = g_pool.tile([P, NFREE], F16, tag="g", name="zd_t")
                nc.vector.tensor_mul(zd_t[:], z_t[:], d_sb[:, mt, nbs])
                ht_t = g_pool.tile([P, NFREE], F16, tag="g", name="ht_t")
                nc.vector.tensor_add(ht_t[:], n_sb[:, mt, nbs], zd_t[:])
                nc.sync.dma_start(outs["htT"][mt][:, nbs], ht_t[:])

            phase(
                BLK, pre_sel(pre3a, pre3b), w["w_iz"], w["w_hz"],
                xa, hb, consume_z, False, seq_last=True)

    nc.finalize()
    return nc


_NC = None


def _get_nc():
    global _NC
    if _NC is None:
        _NC = _build_nc()
    return _NC


def _pack_w(W):
    # W [H, IN] -> [MT, P, KO*P] with W_host[mt, p, ko, m] = W[mt*P+m, ko*P+p]
    W = np.ascontiguousarray(np.asarray(W, dtype=np.float32))
    return np.ascontiguousarray(
        W.reshape(MT, P, KO_IN, P).transpose(0, 3, 2, 1).astype(np.float16)
    ).reshape(MT, P, KO_IN * P)


def _pack_hw(pwa, pwb):
    # packed weights [MT, P, KO*P] x2 -> [P, KO, 2*BLK1, P] in arrival order
    st = np.stack(
        [pwa[:BLK1].reshape(BLK1, P, KO_IN, P),
         pwb[:BLK1].reshape(BLK1, P, KO_IN, P)], axis=1
    )  # [BLK1, 2, P, KO, P]
    return np.ascontiguousarray(st.transpose(2, 3, 0, 1, 4)).reshape(
        P, KO_IN, 2 * BLK1, P)


def _pack_xh(x, h):
    # x,h [BS, D] -> [P, KO, 2, BS] with xh[p, ko, i, b] = (x,h)[b, ko*P+p]
    def t(a):
        return (np.asarray(a, dtype=np.float32).reshape(BS, -1, P)
                .transpose(2, 1, 0).astype(np.float16))
    return np.ascontiguousarray(np.stack([t(x), t(h)], axis=2))


def _pack_b(bvec):
    # b [H] -> [P, MT] with b_host[p, mt] = b[mt*P+p]
    return np.ascontiguousarray(
        np.asarray(bvec, dtype=np.float32).reshape(MT, P).T
    )


def _unpack(arr):
    # [MT, P, BS] fp16 -> [BS, H] fp32
    return np.ascontiguousarray(
        arr.astype(np.float32).transpose(2, 0, 1)
    ).reshape(BS, H)


def kernel(x, h, W_ir, b_ir, W_hr, W_iz, b_iz, W_hz, W_in, b_in, W_hn):
    global LAST_RESULTS
    nc = _get_nc()

    x = np.ascontiguousarray(np.asarray(x, dtype=np.float32))
    h = np.ascontiguousarray(np.asarray(h, dtype=np.float32))

    pw = {
        "w_ir": _pack_w(W_ir), "w_hr": _pack_w(W_hr),
        "w_iz": _pack_w(W_iz), "w_hz": _pack_w(W_hz),
        "w_in": _pack_w(W_in), "w_hn": _pack_w(W_hn),
    }
    shared = {
        **pw,
        "hw": _pack_hw(pw["w_ir"], pw["w_hr"]),
        "b_ir": _pack_b(b_ir), "b_iz": _pack_b(b_iz), "b_in": _pack_b(b_in),
    }
    in_maps = []
    for c in range(N_CORES):
        sl = slice(c * BS, (c + 1) * BS)
        in_maps.append({
            "xh": _pack_xh(x[sl], h[sl]),
            **shared,
        })

    res = run_bass_kernel_spmd(
        nc, in_maps, core_ids=list(range(N_CORES)), trace=TRACE
    )
    LAST_RESULTS = res

    def full(name):
        return np.concatenate(
            [_unpack(res.results[c][name]) for c in range(N_CORES)], axis=0
        )

    return full("htT"), full("rT"), full("zT"), full("nT")
